# revision 37
# baseline (speedup 1.0000x reference)
"""MOELinearDGLFractional Trainium2 kernel.

Data-parallel over systems: 8 cores x 64 systems (512 rows each).

Host precomputes (per core, off the device critical path):
  - x cast to bf16
  - vmoe [128 i', n_sys s, 2 h, 128 o] bf16: per-system mixed expert weights
    (einsum over E on host). The regular Linear weights are stored ONCE
    (linw2 [128, 2, 128]) and addressed by a strided 2-element AP dim in the
    matmul rhs, so they are never replicated in DRAM or SBUF.
  - bias4 [128, 1024] f32, ident bf16

Device pipeline per system s (rows r = 512*s + 4*q + j, q=partition, j=0..3):
  sync DMA x tile [128, 1024] bf16 (6-deep prefetch) and the out stream
  (lagged 8 systems) both on the sync HWDGE queue (no SWDGE => no gpsimd
  queue-drain at the end); vmoe chunks go out on the scalar HWDGE queue so
  x never queues behind them.
  PE transposes (system t) run two systems ahead of the fused matmuls
  (system t-2) so the scalar evac latency is off the critical path.
  Each main matmul streams 256 rhs rows: 128 moe rows (per-system) plus 128
  shared Linear rows via the strided AP.
  DVE adds bias in one [128,1024] op + casts to bf16 into o_sb (6-deep).
Host casts the gathered bf16 output back to fp32.
"""

import sys

sys.path.insert(0, "/opt/trn_rl_repo")

import numpy as np
import ml_dtypes

N_TOTAL = 262144
B = 512
E = 16
I_DIM = 256
O_MOE = 128
O_REG = 128
NCORES = 8
L = 512  # rows per system
NXB = 6  # x_sb prefetch depth
NOB = 6  # o_sb depth
OLAG = 8  # out-DMA issue lag behind x issues on the sync stream

BF16 = ml_dtypes.bfloat16


def _v3_chunks(n_sys):
    """Chunk boundaries for the vmoe load: small leading chunks so mains(0)
    starts early, then 8-system chunks."""
    bounds = [0]
    for b in (1, 2, 4, 8):
        if b < n_sys:
            bounds.append(b)
    while bounds[-1] + 16 < n_sys:
        bounds.append(bounds[-1] + 16)
    return [(bounds[i], bounds[i + 1] if i + 1 < len(bounds) else n_sys)
            for i in range(len(bounds))]


def build_program(n_sys):
    import concourse.bass as bass
    import concourse.mybir as mybir

    f32 = mybir.dt.float32
    bf16 = mybir.dt.bfloat16
    rows = n_sys * L
    chunks = _v3_chunks(n_sys)
    nch = len(chunks)
    chunk_starts = {lo: k for k, (lo, hi) in enumerate(chunks)}

    nc = bass.Bass()
    x = nc.declare_dram_parameter("x", [rows, I_DIM], bf16, isOutput=False)
    vmoe_d = nc.declare_dram_parameter(
        "vmoe", [128, n_sys * 256], bf16, isOutput=False
    )
    linw_d = nc.declare_dram_parameter("linw2", [128, 256], bf16, isOutput=False)
    bias_d = nc.declare_dram_parameter("bias4", [128, 1024], f32, isOutput=False)
    ident_d = nc.declare_dram_parameter("ident", [128, 128], bf16, isOutput=False)
    out = nc.declare_dram_parameter("out", [rows, 256], bf16, isOutput=True)

    xv = x.rearrange("(s q j) m -> s q (j m)", q=128, j=4)
    ov = out.rearrange("(s q j) m -> s q (j m)", q=128, j=4)

    from contextlib import ExitStack

    with ExitStack() as ctx:
        en = ctx.enter_context
        # blocks 0..2*n_sys-1: moe (blk = 2s+h); blocks 2*n_sys+h: shared Linear
        vall = en(nc.sbuf_tensor("vall_sb", [128, 2 * n_sys + 2, 128], bf16))
        bias4 = en(nc.sbuf_tensor("bias4_sb", [128, 1024], f32))
        ident = en(nc.sbuf_tensor("ident_sb", [128, 128], bf16))
        x_sb = [en(nc.sbuf_tensor(f"x_sb{i}", [128, 1024], bf16)) for i in range(NXB)]
        xt_sb = [en(nc.sbuf_tensor(f"xt_sb{i}", [128, 1024], bf16)) for i in range(2)]
        o_sb = [en(nc.sbuf_tensor(f"o_sb{i}", [128, 1024], bf16)) for i in range(NOB)]
        # PSUM: xtp 2x1 bank (bf16), outp 3x2 banks (f32) = 8 banks
        xtp = [en(nc.psum_tensor(f"xtp{i}", [128, 1024], bf16)) for i in range(2)]
        outp = [en(nc.psum_tensor(f"outp{i}", [128, 1024], f32)) for i in range(3)]

        sem_names = ["idc", "cst", "xp", "xt", "mm", "dve"]
        sems = {n: en(nc.semaphore(n)) for n in sem_names}
        idc_s, cst_s, xp_s, xt_s, mm_s, dve_s = (sems[n] for n in sem_names)
        xin = [en(nc.semaphore(f"xin{i}")) for i in range(NXB)]
        vch = [en(nc.semaphore(f"vch{k}")) for k in range(nch)]
        dout = [en(nc.semaphore(f"dout{i}")) for i in range(NOB)]

        block = en(nc.Block())

        @block.sync
        def _(sync):
            # x in-stream and out-stream both ride the sync HWDGE queue
            # (no SWDGE => no 6us gpsimd queue drain at the end); vmoe goes
            # out on the scalar HWDGE queue so x never queues behind it.
            sync.dma_start(out=ident[:], in_=ident_d[:]).then_inc(idc_s, 16)
            for s in range(n_sys):
                if s >= NXB:
                    sync.wait_ge(xp_s, s - (NXB - 1))
                sync.dma_start(out=x_sb[s % NXB][:], in_=xv[s]).then_inc(
                    xin[s % NXB], 16
                )
                if s == 0:
                    sync.dma_start(out=bias4[:], in_=bias_d[:]).then_inc(cst_s, 16)
                    sync.dma_start(
                        out=vall[:, 2 * n_sys : 2 * n_sys + 2, :], in_=linw_d[:]
                    ).then_inc(cst_s, 16)
                if s >= OLAG:
                    so = s - OLAG
                    sync.wait_ge(dve_s, so + 1)
                    sync.dma_start(out=ov[so], in_=o_sb[so % NOB][:]).then_inc(
                        dout[so % NOB], 16
                    )
            for so in range(max(0, n_sys - OLAG), n_sys):
                sync.wait_ge(dve_s, so + 1)
                sync.dma_start(out=ov[so], in_=o_sb[so % NOB][:]).then_inc(
                    dout[so % NOB], 16
                )
            for i in range(NOB):
                cnt = (n_sys - i + NOB - 1) // NOB
                if cnt:
                    sync.wait_ge(dout[i], 16 * cnt)

        @block.tensor
        def _(tensor):
            def mains(m):
                tensor.wait_ge(xt_s, m + 1)
                if m == 0:
                    tensor.wait_ge(cst_s, 32)  # bias4+linw2 loaded
                if m in chunk_starts:
                    tensor.wait_ge(vch[chunk_starts[m]], 16)
                if m >= 3:
                    tensor.wait_ge(dve_s, m - 2)
                step = 2 * (n_sys - m)
                for j in range(4):
                    for h in range(2):
                        blk = 2 * m + h
                        inst = nc.tensor.matmul(
                            outp[m % 3][:, j * 256 : (j + 1) * 256],
                            xt_sb[m % 2][:, (2 * j + h) * 128 : (2 * j + h + 1) * 128],
                            vall[:, blk : blk + step + 1 : step, :],
                            start=(h == 0),
                            stop=(h == 1),
                        )
                    if j == 1 or j == 3:
                        inst.then_inc(mm_s, 1)

            def transposes(t):
                tensor.wait_ge(xin[t % NXB], 16 * (t // NXB + 1))
                if t == 0:
                    tensor.wait_ge(idc_s, 16)  # ident loaded
                if t >= 2:
                    tensor.wait_ge(xt_s, t - 1)
                for j in range(4):
                    for h in range(2):
                        k = 2 * j + h
                        inst = nc.tensor.transpose(
                            xtp[t % 2][:, k * 128 : (k + 1) * 128],
                            x_sb[t % NXB][:, j * 256 + h * 128 : j * 256 + h * 128 + 128],
                            ident[:],
                        )
                inst.then_inc(xp_s, 1)

            # transposes run two systems ahead of mains
            for t in range(n_sys):
                transposes(t)
                if t >= 2:
                    mains(t - 2)
            mains(n_sys - 2)
            mains(n_sys - 1)

        @block.scalar
        def _(scalar):
            def v3_dma(k):
                lo, hi = chunks[k]
                scalar.dma_start(
                    out=vall[:, 2 * lo : 2 * hi, :],
                    in_=vmoe_d[:, lo * 256 : hi * 256],
                ).then_inc(vch[k], 16)

            v3_dma(0)
            if nch > 1:
                v3_dma(1)
            for s in range(n_sys):
                scalar.wait_ge(xp_s, s + 1)
                if s >= 2:
                    scalar.wait_ge(mm_s, 2 * s - 2)
                nc.scalar.copy(out=xt_sb[s % 2][:], in_=xtp[s % 2][:]).then_inc(
                    xt_s, 1
                )
                if s + 2 < nch:
                    v3_dma(s + 2)

        @block.vector
        def _(vector):
            for s in range(n_sys):
                if s == 0:
                    vector.wait_ge(cst_s, 32)  # bias4+linw2 loaded
                vector.wait_ge(mm_s, 2 * s + 2)
                if s >= NOB:
                    vector.wait_ge(dout[s % NOB], 16 * (s // NOB))
                nc.vector.tensor_add(
                    o_sb[s % NOB][:], outp[s % 3][:], bias4[:]
                ).then_inc(dve_s, 1)

    return nc


def _host_inputs(x_bf16, coeff, moe_weights, moe_bias, lin_weight, lin_bias, n_sys, core):
    """Build per-core in_map. x_bf16 is the FULL x already cast to bf16."""
    b0 = core * n_sys
    xs = np.ascontiguousarray(x_bf16[b0 * L : (b0 + n_sys) * L])

    Wr = np.asarray(moe_weights, dtype=np.float32).reshape(E, O_MOE, 2, 128)
    flat = Wr.reshape(E, -1)  # [E, o*h*i']
    cm = np.asarray(coeff, dtype=np.float32)[b0 : b0 + n_sys]  # [n_sys, E]
    wm = cm @ flat  # [n_sys, o*h*i']
    wm4 = wm.reshape(n_sys, O_MOE, 2, 128).transpose(3, 0, 2, 1)  # [i', s, h, o]
    vmoe = np.ascontiguousarray(wm4.astype(BF16)).reshape(128, -1)
    linw2 = np.ascontiguousarray(
        np.asarray(lin_weight, dtype=np.float32)
        .reshape(O_REG, 2, 128)
        .transpose(2, 1, 0)  # [i', h, o']
        .astype(BF16)
        .reshape(128, 256)
    )

    bias_cat = np.concatenate([np.asarray(moe_bias), np.asarray(lin_bias)]).astype(
        np.float32
    )
    bias4 = np.tile(bias_cat, (128, 4))
    ident = np.eye(128, dtype=BF16)
    return {
        "x": xs,
        "vmoe": vmoe,
        "linw2": linw2,
        "bias4": bias4,
        "ident": ident,
    }


_CACHE = {}


def kernel(
    x,
    expert_mixing_coefficients,
    routing_idxs,
    moe_weights,
    moe_bias,
    lin_weight,
    lin_bias,
    trace=False,
):
    from concourse.bass_utils import run_bass_kernel_spmd

    n_sys = B // NCORES
    if "nc" not in _CACHE:
        _CACHE["nc"] = build_program(n_sys)
    nc = _CACHE["nc"]
    x_bf16 = np.asarray(x).astype(BF16)
    in_maps = [
        _host_inputs(
            x_bf16, expert_mixing_coefficients, moe_weights, moe_bias, lin_weight,
            lin_bias, n_sys, c,
        )
        for c in range(NCORES)
    ]
    res = run_bass_kernel_spmd(nc, in_maps, list(range(NCORES)), trace=trace)
    outs = [res.results[c]["out"] for c in range(NCORES)]
    full = np.concatenate(outs, axis=0).astype(np.float32)
    if trace:
        return full, res
    return full
